# revision 1
# baseline (speedup 1.0000x reference)
"""Trainium2 Bass kernel for nn_Half_Graph (GNN message passing block).

Data-parallel over batch: core b processes image b (B=8 across 8 cores).

Per-core layout ("planar G=6"): the 36864-pixel image plane is split into
6 groups of 6144 pixels; a 10-channel tensor occupies 60 SBUF partitions
(partition 10*g + c <-> channel c, pixel group g), padded with 4 zero
rows to 64. Pairs / 20-channel entities use two such 64-row halves at
partitions [0:64] and [64:128] (matmul base partitions must be 0/32/64).
Rows 60..63 / 124..127 are always 0.

All convs are 1x1 so every conv is a matmul over the channel dim with a
block-diagonal (per-group) stationary matrix, BN folded into weights and
bias. Every matmul uses M=128 stationaries (zero cols where inactive) so
PSUM tiles are always fully written. Attention maps are broadcast across
channels with a ones-pattern stationary on the tensor engine (which also
sums the p_att planes for free). The inter-block message sum runs as
identity-matmul accumulation in PSUM. GRU output uses
out = h + u * (c - h).

Host side pre-transposes image planes into the planar layout (cheap, not
part of the measured device time) so every DMA is a plain 2D slice.
"""

import sys

for _p in ("/opt/trn_rl_repo", "/root/.axon_site/_ro/trn_rl_repo"):
    if _p not in sys.path:
        sys.path.insert(0, _p)

import numpy as np

import concourse.bass as bass
import concourse.bacc as bacc
import concourse.mybir as mybir
from concourse.tile import TileContext

F32 = mybir.dt.float32
BF16 = mybir.dt.bfloat16
AL = mybir.AluOpType
AF = mybir.ActivationFunctionType

B = 8
HD = 10
HW = 192 * 192          # 36864 pixels
G = 6                   # pixel groups
GP = HW // G            # 6144 pixels per group
CW = 1024               # chunk width (columns per group per chunk)
NCHUNK = GP // CW       # 6 chunks
EPS = 1e-5
H1 = 64                 # partition offset of half 1

# stationary matrix indices
(S_A12, S_UL, S_SPARE,
 S_DW1A0, S_DW1B0, S_UW1A0, S_UW1B0, S_LW1A0, S_LW1B0,
 S_DW1A6, S_DW1B6, S_UW1A6, S_UW1B6, S_LW1A6, S_LW1B6,
 S_DW2H0, S_DW2H1, S_UW2H0, S_UW2H1, S_LW2H0, S_LW2H1,
 S_I0, S_I3,
 S_GUWG_R, S_GLWG_R, S_GUWG_U, S_GLWG_U, S_GUWC, S_GLWC) = range(29)
NS = 29

# bias vector indices
(BV_D1, BV_U1, BV_L1, BV_Z0, BV_Z1, BV_Z3, BV_R, BV_U, BV_C) = range(9)
NB = 9

# comp block processing order and Z-pair mapping:
#   Z1 = z_c0 (+) z_c4 ; Z2 = z_c1 (+) z_c5 ; Z3 = z_c2 (+) z_c3
BLOCK_ORDER = [0, 4, 1, 5, 2, 3]
BLOCK_ZPAIR = {0: (1, 0), 4: (1, 1), 1: (2, 0), 5: (2, 1), 2: (3, 0), 3: (3, 1)}
# conv2 stationary per (upper, zhalf)
W2_STAT = {(True, 0): S_UW2H0, (True, 1): S_UW2H1,
           (False, 0): S_LW2H0, (False, 1): S_LW2H1}


def _build_nc():
    nc = bacc.Bacc(trn_type="TRN2")

    # image tensors arrive host-pretransposed to padded planar layout:
    # row 10*g + c <-> (channel c, pixel group g); rows 60..63 zero
    xf2 = nc.declare_dram_parameter("xf2", [128, GP], F32, isOutput=False)
    xh = nc.declare_dram_parameter("xh", [128, GP], F32, isOutput=False)
    xhB = nc.declare_dram_parameter("xhB", [128, GP], BF16, isOutput=False)
    xp = nc.declare_dram_parameter("xp", [4, 128, GP], F32, isOutput=False)
    hatt = nc.declare_dram_parameter("hatt", [12, GP], BF16, isOutput=False)
    patt = nc.declare_dram_parameter("patt", [36, GP], BF16, isOutput=False)
    smats = nc.declare_dram_parameter("smats", [NS, 128, 128], BF16, isOutput=False)
    bvecs = nc.declare_dram_parameter("bvecs", [128, NB], F32, isOutput=False)
    out = nc.declare_dram_parameter("out", [2, 60, GP], F32, isOutput=True)

    def csl(t, j):
        return t[:, j * CW:(j + 1) * CW]

    with TileContext(nc) as tc:
        with (
            tc.tile_pool(name="const", bufs=1) as cpool,
            tc.tile_pool(name="xin", bufs=2) as xin,
            tc.tile_pool(name="xin1", bufs=2) as xin1,
            tc.tile_pool(name="attp", bufs=2) as attp,
            tc.tile_pool(name="pmul", bufs=2) as pmul,
            tc.tile_pool(name="cat", bufs=2) as catp,
            tc.tile_pool(name="hmid", bufs=4) as hpool,
            tc.tile_pool(name="zmid", bufs=5) as zpool,
            tc.tile_pool(name="gmid", bufs=2) as gpool,
            tc.tile_pool(name="gmid1", bufs=1) as gpool1,
            tc.tile_pool(name="psum", bufs=4, space="PSUM") as pp,
        ):
            smt = cpool.tile([128, NS * 128], BF16)
            for n in range(NS):
                nc.sync.dma_start(out=smt[:, n * 128:(n + 1) * 128],
                                  in_=smats[n, :, :])
            bv = cpool.tile([128, NB], F32)
            nc.sync.dma_start(out=bv[:, :], in_=bvecs[:, :])

            def stat(i, K, base=0):
                return smt[base:base + K, i * 128:(i + 1) * 128]

            def mm(psum_tile, s_idx, K, rhs_ap, start, stop, base=0):
                # one logical pass = 512-col matmuls covering CW columns
                lhsT = stat(s_idx, K, base)
                for s in range(0, CW, 512):
                    nc.tensor.matmul(
                        psum_tile[0:128, s:s + 512],
                        lhsT,
                        rhs_ap[:, s:s + 512],
                        start=start, stop=stop)

            def bias(k):
                return bv[0:128, k:k + 1]

            for j in range(NCHUNK):
                # ---------------- loads ----------------
                def pair_load(pool, tag, srcpair):
                    t = pool.tile([128, CW], srcpair.dtype, tag=tag, name=tag)
                    nc.sync.dma_start(out=t[:, :],
                                      in_=srcpair[:, j * CW:(j + 1) * CW])
                    return t

                xhd = pair_load(xin, "xhd", xh)
                xhdB = pair_load(xin, "xhdB", xhB)
                xfd = pair_load(xin1, "xfd", xf2)
                xpd = []
                for pr in range(4):
                    t = xin1.tile([128, CW], F32, tag=f"xpd{pr}", name=f"xpd{pr}")
                    rows = 128 if pr < 2 else 64
                    nc.sync.dma_start(out=t[0:rows, :],
                                      in_=xp[pr, 0:rows, j * CW:(j + 1) * CW])
                    xpd.append(t)
                attA = attp.tile([36, CW], BF16, tag="attA")
                nc.sync.dma_start(out=attA[:, :], in_=csl(patt, j))
                att12 = attp.tile([12, CW], BF16, tag="att12")
                nc.sync.dma_start(out=att12[:, :], in_=csl(hatt, j))
                # GRU Wg concat tiles: bottom halves = xh (HBM re-read)
                catWg_u = catp.tile([128, CW], BF16, tag="catWg_u")
                nc.sync.dma_start(out=catWg_u[64:128, :], in_=xhB[0:64, j * CW:(j + 1) * CW])
                catWg_l = catp.tile([128, CW], BF16, tag="catWg_l")
                nc.sync.dma_start(out=catWg_l[64:128, :], in_=xhB[64:128, j * CW:(j + 1) * CW])
                catWc_u = catp.tile([128, CW], BF16, tag="catWc_u")
                catWc_l = catp.tile([128, CW], BF16, tag="catWc_l")

                # ------------- attention broadcast (PE) -------------
                p_ul = pp.tile([128, CW], F32, tag="ps")
                mm(p_ul, S_UL, 36, attA[0:36, :], True, True)
                p_a12 = pp.tile([128, CW], F32, tag="ps")
                mm(p_a12, S_A12, 12, att12[0:12, :], True, True)

                # ------------- premultiplies (DVE) -------------
                xpm = []
                for pr in range(4):
                    rows = 128 if pr < 2 else 64
                    t = pmul.tile([128, CW], BF16, tag=f"xpm{pr}", name=f"xpm{pr}")
                    nc.vector.tensor_tensor(t[0:rows, :], xpd[pr][0:rows, :],
                                            p_ul[0:rows, :], AL.mult)
                    xpm.append(t)
                xfm = pmul.tile([128, CW], BF16, tag="xfm")
                nc.vector.tensor_tensor(xfm[0:128, :], xfd[0:128, :],
                                        p_a12[0:128, :], AL.mult)

                # ------------- decomposition blocks -------------
                p_du = pp.tile([128, CW], F32, tag="ps")
                mm(p_du, S_DW1A0, 60, xfm[0:60, :], True, False)
                mm(p_du, S_DW1B0, 60, xhdB[0:60, :], False, True)
                H_du = hpool.tile([128, CW], BF16, tag="H")
                nc.scalar.activation(H_du[0:128, :], p_du[0:128, :], AF.Relu,
                                     bias=bias(BV_D1))
                p_dl = pp.tile([128, CW], F32, tag="ps")
                mm(p_dl, S_DW1A6, 60, xfm[H1:H1 + 60, :], True, False, base=H1)
                mm(p_dl, S_DW1B6, 60, xhdB[H1:H1 + 60, :], False, True, base=H1)
                H_dl = hpool.tile([128, CW], BF16, tag="H")
                nc.scalar.activation(H_dl[0:128, :], p_dl[0:128, :], AF.Relu,
                                     bias=bias(BV_D1))
                Z0 = pp.tile([128, CW], F32, tag="ps")
                mm(Z0, S_DW2H0, 128, H_du[0:128, :], True, False)
                mm(Z0, S_DW2H1, 128, H_dl[0:128, :], False, True)
                z0t = zpool.tile([128, CW], BF16, tag="zt")
                nc.vector.tensor_scalar(z0t[0:128, :], Z0[0:128, :],
                                        bias(BV_Z0), 0.0, AL.add, AL.max)

                # ------------- composition blocks -------------
                zpsum = {}
                zt = {}
                for i in BLOCK_ORDER:
                    up = i < 4
                    xh_sl = xhdB[0:60, :] if up else xhdB[H1:H1 + 60, :]
                    sa, ab = (S_UW1A0, 0) if up else (S_LW1A6, H1)
                    t = xpm[i] if up else xpm[i - 4]
                    if up:
                        xpm_sl, sb, bb = t[0:60, :], S_UW1B0, 0
                    else:
                        xpm_sl, sb, bb = t[H1:H1 + 60, :], S_LW1B6, H1
                    p_c = pp.tile([128, CW], F32, tag="ps", name=f"pc{i}")
                    mm(p_c, sa, 60, xh_sl, True, False, base=ab)
                    mm(p_c, sb, 60, xpm_sl, False, True, base=bb)
                    H_c = hpool.tile([128, CW], BF16, tag="H", name=f"Hc{i}")
                    nc.scalar.activation(H_c[0:128, :], p_c[0:128, :], AF.Relu,
                                         bias=bias(BV_U1 if up else BV_L1))
                    zi, half = BLOCK_ZPAIR[i]
                    if zi not in zpsum:
                        zpsum[zi] = pp.tile([128, CW], F32, tag="ps", name=f"zp{zi}")
                    mm(zpsum[zi], W2_STAT[(up, half)], 128, H_c[0:128, :],
                       half == 0, half == 1)
                    if half == 1:
                        bz = BV_Z1 if zi in (1, 2) else BV_Z3
                        zt[zi] = zpool.tile([128, CW], BF16, tag="zt", name=f"zt{zi}")
                        nc.vector.tensor_scalar(zt[zi][0:128, :], zpsum[zi][0:128, :],
                                                bias(bz), 0.0, AL.add, AL.max)

                # ------------- message sum (PE identity) -------------
                p_msg = pp.tile([128, CW], F32, tag="ps")
                mm(p_msg, S_I0, 128, z0t[0:128, :], True, False)
                mm(p_msg, S_I0, 128, zt[1][0:128, :], False, False)
                mm(p_msg, S_I0, 128, zt[2][0:128, :], False, False)
                mm(p_msg, S_I3, 128, zt[3][0:128, :], False, True)
                nc.scalar.activation(catWg_u[0:64, :], p_msg[0:64, :], AF.Copy)
                nc.scalar.activation(catWg_l[0:64, :], p_msg[H1:H1 + 64, :], AF.Copy)
                nc.vector.tensor_copy(catWc_u[0:64, :], p_msg[0:64, :])
                nc.vector.tensor_copy(catWc_l[0:64, :], p_msg[H1:H1 + 64, :])

                # ------------- GRU gates -------------
                p_r = pp.tile([128, CW], F32, tag="ps")
                mm(p_r, S_GUWG_R, 128, catWg_u[0:128, :], True, False)
                mm(p_r, S_GLWG_R, 128, catWg_l[0:128, :], False, True)
                p_u = pp.tile([128, CW], F32, tag="ps")
                mm(p_u, S_GUWG_U, 128, catWg_u[0:128, :], True, False)
                mm(p_u, S_GLWG_U, 128, catWg_l[0:128, :], False, True)
                Rt = gpool.tile([128, CW], BF16, tag="Rt")
                nc.scalar.activation(Rt[0:128, :], p_r[0:128, :], AF.Sigmoid,
                                     bias=bias(BV_R))
                Ut = gpool.tile([128, CW], F32, tag="Ut")
                nc.scalar.activation(Ut[0:128, :], p_u[0:128, :], AF.Sigmoid,
                                     bias=bias(BV_U))

                # rh = r * h into Wc concat bottoms
                nc.gpsimd.tensor_tensor(catWc_u[H1:H1 + 64, :], Rt[0:64, :],
                                        xhdB[0:64, :], AL.mult)
                nc.gpsimd.tensor_tensor(catWc_l[H1:H1 + 64, :], Rt[H1:H1 + 64, :],
                                        xhdB[H1:H1 + 64, :], AL.mult)

                p_cc = pp.tile([128, CW], F32, tag="ps")
                mm(p_cc, S_GUWC, 128, catWc_u[0:128, :], True, False)
                mm(p_cc, S_GLWC, 128, catWc_l[0:128, :], False, True)
                Ct = gpool.tile([128, CW], F32, tag="Ct")
                nc.scalar.activation(Ct[0:128, :], p_cc[0:128, :], AF.Tanh,
                                     bias=bias(BV_C))

                # ------------- GRU combine: out = h + u*(c - h) -------------
                Dt = gpool1.tile([128, CW], F32, tag="Dt")
                nc.gpsimd.tensor_tensor(Dt[0:128, :], Ct[0:128, :],
                                        xhd[0:128, :], AL.subtract)
                Et = gpool1.tile([128, CW], F32, tag="Et")
                nc.gpsimd.tensor_tensor(Et[0:128, :], Ut[0:128, :],
                                        Dt[0:128, :], AL.mult)
                outd = gpool.tile([128, CW], F32, tag="outd")
                nc.gpsimd.tensor_tensor(outd[0:128, :], xhd[0:128, :],
                                        Et[0:128, :], AL.add)

                # ------------- store -------------
                nc.sync.dma_start(out=csl(out[0], j), in_=outd[0:60, :])
                nc.sync.dma_start(out=csl(out[1], j), in_=outd[H1:H1 + 60, :])

    nc.compile()
    return nc


def _fold(W, p):
    g, b, m, v = p[0], p[1], p[2], p[3]
    s = g / np.sqrt(v + EPS)
    return (s[:, None] * W).astype(np.float32), (b - m * s).astype(np.float32)


def _build_params(dW1, dbn1, dW2, dbn2, uW1, ubn1, uW2, ubn2,
                  lW1, lbn1, lW2, lbn2, guWg, gubg, guWc, gubc,
                  glWg, glbg, glWc, glbc):
    dW1f, bd1 = _fold(dW1, dbn1)
    dW2f, bd2 = _fold(dW2, dbn2)
    uW1f, bu1 = _fold(uW1, ubn1)
    uW2f, bu2 = _fold(uW2, ubn2)
    lW1f, bl1 = _fold(lW1, lbn1)
    lW2f, bl2 = _fold(lW2, lbn2)

    S = np.zeros((NS, 128, 128), np.float32)
    ci = np.arange(HD)
    # RC[g, c] = row/col index of (group g, channel c) in a 64-row half
    RC = np.stack([10 * g + ci for g in range(G)])  # [6, 10]

    # attention broadcasts: A12 = h_att1(top)/h_att2(bottom);
    # UL = sum p_att1..4 (top) / sum p_att5..6 (bottom)
    for g in range(G):
        S[S_A12, g, RC[g]] = 1.0
        S[S_A12, 6 + g, H1 + RC[g]] = 1.0
        for k in range(4):
            S[S_UL, 6 * k + g, RC[g]] = 1.0
        for k in (4, 5):
            S[S_UL, 6 * k + g, H1 + RC[g]] = 1.0

    def conv1_stat(i0, i6, Wf, in_off):
        # rows (moving ch ci): base variant handles the 64 offset
        # cols: out ch co -> 10g+co ; out ch 10+co -> 64+10g+co
        for g in range(G):
            S[np.ix_([i0], RC[g], RC[g])] = Wf[0:10, in_off:in_off + 10].T[None]
            S[np.ix_([i0], RC[g], H1 + RC[g])] = Wf[10:20, in_off:in_off + 10].T[None]
            S[np.ix_([i6], H1 + RC[g], RC[g])] = Wf[0:10, in_off:in_off + 10].T[None]
            S[np.ix_([i6], H1 + RC[g], H1 + RC[g])] = Wf[10:20, in_off:in_off + 10].T[None]

    conv1_stat(S_DW1A0, S_DW1A6, dW1f, 0)    # xf*att part (concat ch 0..9)
    conv1_stat(S_DW1B0, S_DW1B6, dW1f, 10)   # xh part
    conv1_stat(S_UW1A0, S_UW1A6, uW1f, 0)    # xh part (first in concat)
    conv1_stat(S_UW1B0, S_UW1B6, uW1f, 10)   # xp*att part
    conv1_stat(S_LW1A0, S_LW1A6, lW1f, 0)
    conv1_stat(S_LW1B0, S_LW1B6, lW1f, 10)

    def conv2_stat(ih0, ih1, Wf):
        # moving rows: H planar [0:60]=in ch 0..9, [64:124]=in ch 10..19
        # cols: out ch co -> 10g+co (H0 variant) or 64+10g+co (H1 variant)
        for g in range(G):
            for idx, off in ((ih0, 0), (ih1, H1)):
                S[np.ix_([idx], RC[g], off + RC[g])] = Wf[:, 0:10].T[None]
                S[np.ix_([idx], H1 + RC[g], off + RC[g])] = Wf[:, 10:20].T[None]

    conv2_stat(S_DW2H0, S_DW2H1, dW2f)
    conv2_stat(S_UW2H0, S_UW2H1, uW2f)
    conv2_stat(S_LW2H0, S_LW2H1, lW2f)

    for g in range(G):
        S[S_I0, RC[g], RC[g]] = 1.0
        S[S_I0, H1 + RC[g], H1 + RC[g]] = 1.0
        S[S_I3, RC[g], RC[g]] = 1.0
        S[S_I3, H1 + RC[g], RC[g]] = 1.0

    def gru_stat(idx, W, out_rows, off):
        # concat rows: [0:60]=msg (in ch 0..9), [64:124]=h or rh (in ch 10..19)
        for g in range(G):
            S[np.ix_([idx], RC[g], off + RC[g])] = W[out_rows, 0:10].T[None]
            S[np.ix_([idx], H1 + RC[g], off + RC[g])] = W[out_rows, 10:20].T[None]

    gru_stat(S_GUWG_R, guWg, slice(0, 10), 0)
    gru_stat(S_GLWG_R, glWg, slice(0, 10), H1)
    gru_stat(S_GUWG_U, guWg, slice(10, 20), 0)
    gru_stat(S_GLWG_U, glWg, slice(10, 20), H1)
    gru_stat(S_GUWC, guWc, slice(0, 10), 0)
    gru_stat(S_GLWC, glWc, slice(0, 10), H1)

    bvec = np.zeros((128, NB), np.float32)

    def setb(col, top, bot):
        for g in range(G):
            bvec[RC[g], col] = top
            bvec[H1 + RC[g], col] = bot

    setb(BV_D1, bd1[0:10], bd1[10:20])
    setb(BV_U1, bu1[0:10], bu1[10:20])
    setb(BV_L1, bl1[0:10], bl1[10:20])
    setb(BV_Z0, bd2, bd2)
    setb(BV_Z1, bu2, bl2)
    setb(BV_Z3, bu2, bu2)
    setb(BV_R, gubg[0:10], glbg[0:10])
    setb(BV_U, gubg[10:20], glbg[10:20])
    setb(BV_C, gubc, glbc)

    return S, bvec


_NC_CACHE = None


def _get_nc():
    global _NC_CACHE
    if _NC_CACHE is None:
        _NC_CACHE = _build_nc()
    return _NC_CACHE


def _planar(a):
    # [..., HD, H, W] -> [..., 64, GP] zero-padded planar
    lead = a.shape[:-3]
    a = np.asarray(a, np.float32).reshape(lead + (HD, G, GP))
    a = np.moveaxis(a, -2, -3)          # [..., G, HD, GP]
    a = a.reshape(lead + (60, GP))
    pad = np.zeros(lead + (4, GP), np.float32)
    return np.ascontiguousarray(np.concatenate([a, pad], axis=-2))


def _att_planar(a):
    # [K, H, W] -> [6K, GP]: row 6*k + g
    K = a.shape[0]
    return np.ascontiguousarray(np.asarray(a, np.float32).reshape(K * G, GP))


def _unplanar(a):
    # [..., 60, GP] -> [..., HD, H, W]
    lead = a.shape[:-2]
    a = a.reshape(lead + (G, HD, GP))
    a = np.moveaxis(a, -3, -2)          # [..., HD, G, GP]
    return a.reshape(lead + (HD, 192, 192))


BF_NP = mybir.dt.np(mybir.dt.bfloat16)


def make_in_maps(xf, xh, xp, h_att, p_att, smats, bvecs):
    smatsB = smats.astype(BF_NP)
    in_maps = []
    for b in range(B):
        xhP = _planar(xh[:, b])           # [2, 64, GP]
        xhPair = np.ascontiguousarray(xhP.reshape(128, GP))
        xfP = _planar(xf[b])              # [64, GP]
        xpP = _planar(xp[:, b])           # [6, 64, GP]
        zz = np.zeros((64, GP), np.float32)
        xpPairs = np.ascontiguousarray(np.stack([
            np.concatenate([xpP[0], xpP[4]], axis=0),
            np.concatenate([xpP[1], xpP[5]], axis=0),
            np.concatenate([xpP[2], zz], axis=0),
            np.concatenate([xpP[3], zz], axis=0)]))
        in_maps.append(dict(
            xf2=np.ascontiguousarray(np.concatenate([xfP, xfP], axis=0)),
            xh=xhPair,
            xhB=xhPair.astype(BF_NP),
            xp=xpPairs,
            hatt=_att_planar(h_att[1:3, b, 0]).astype(BF_NP),
            patt=_att_planar(p_att[1:7, b, 0]).astype(BF_NP),
            smats=smatsB,
            bvecs=bvecs,
        ))
    return in_maps


def kernel(xf, xh, xp, h_att, p_att,
           dW1, dbn1, dW2, dbn2,
           uW1, ubn1, uW2, ubn2,
           lW1, lbn1, lW2, lbn2,
           guWg, gubg, guWc, gubc,
           glWg, glbg, glWc, glbc,
           _trace=False):
    from concourse.bass_utils import run_bass_kernel_spmd

    args = [np.asarray(a, dtype=np.float32) for a in
            (dW1, dbn1, dW2, dbn2, uW1, ubn1, uW2, ubn2,
             lW1, lbn1, lW2, lbn2, guWg, gubg, guWc, gubc,
             glWg, glbg, glWc, glbc)]
    smats, bvecs = _build_params(*args)
    in_maps = make_in_maps(np.asarray(xf, np.float32), np.asarray(xh, np.float32),
                           np.asarray(xp, np.float32),
                           np.asarray(h_att, np.float32),
                           np.asarray(p_att, np.float32), smats, bvecs)

    nc = _get_nc()
    res = run_bass_kernel_spmd(nc, in_maps, core_ids=list(range(B)),
                               trace=_trace)
    out = np.empty((2, B, HD, 192, 192), np.float32)
    for b in range(B):
        out[:, b] = _unplanar(res.results[b]["out"])
    if _trace:
        return out, res
    return out



# revision 11
# speedup vs baseline: 1.4091x; 1.4091x over previous
"""Trainium2 Bass kernel for nn_Half_Graph (GNN message passing block).

Data-parallel over batch: core b processes image b (B=8 across 8 cores).

Planar layout: SBUF partition 10*g + c <-> (channel c, pixel group g) with
G=6 groups of 6144 pixels; a 10-channel tensor occupies 60 partitions.
128-partition tiles hold two such 60-row halves at [0:60] and [64:124].

All convs are 1x1 -> matmuls with block-diagonal stationaries. Host builds
bf16 concat tensors [first; second] per conv block so each conv1 is a
SINGLE matmul pass (K=128). Partition row 60 of every concat tensor is a
constant 1.0, and stationary row 60 carries the folded BN / GRU biases, so
biases ride along in the matmul for free. The attention premultiply reads
a PE-broadcast copy of the attention maps (computed once for all chunks up
front). The inter-block message sum is a relu+add chain on Pool/DVE
reading conv2 PSUM tiles directly. GRU output uses out = h + u*(c - h).

Host side pre-transposes/concats/casts (cheap, not part of measured
device time) so every DMA is a plain 2D bf16 slice.
"""

import sys

for _p in ("/opt/trn_rl_repo", "/root/.axon_site/_ro/trn_rl_repo"):
    if _p not in sys.path:
        sys.path.insert(0, _p)

import numpy as np

import concourse.bass as bass
import concourse.bacc as bacc
import concourse.mybir as mybir
from concourse.tile import TileContext

F32 = mybir.dt.float32
BF16 = mybir.dt.bfloat16
AL = mybir.AluOpType
AF = mybir.ActivationFunctionType

B = 8
HD = 10
G = 6
HW = 192 * 192          # 36864 pixels
GP = HW // G            # 6144 pixels per group
CW = 1024               # chunk width (columns per group per chunk)
NCHUNK = GP // CW       # 6 chunks
EPS = 1e-5

NSTAT = 16
# S_BU broadcast: [h_att1 (rows 0:60); sum p_att1..4 (rows 64:124)]
# S_BL broadcast: [h_att2 (rows 0:60); sum p_att5..6 (rows 64:124)]
(S_BU, S_BL, S_CD, S_CU, S_CL,
 S_ZDA, S_ZDB, S_ZUA, S_ZUB, S_ZLB,
 S_GRM, S_GRH, S_GUM, S_GUH, S_GCM, S_GCRH) = range(NSTAT)

# conv block schedule: pairs of (cat-tile key, conv1 stat, conv2 stat).
# Each pair accumulates into one Z psum: first element -> z cols [0:60],
# second -> z cols [64:124]. Pairs 3/4 are single (odd number of upper z's).
PAIRS = [
    [("d0", S_CD, S_ZDA), ("d1", S_CD, S_ZDB)],
    [("c0", S_CU, S_ZUA), ("c4", S_CL, S_ZLB)],
    [("c1", S_CU, S_ZUA), ("c5", S_CL, S_ZLB)],
    [("c2", S_CU, S_ZUA)],
    [("c3", S_CU, S_ZUA)],
]


def _build_nc():
    nc = bacc.Bacc(trn_type="TRN2")

    catc = nc.declare_dram_parameter("catc", [6, 128, GP], BF16, isOutput=False)
    catd = nc.declare_dram_parameter("catd", [2, 128, GP], BF16, isOutput=False)
    xhbd = nc.declare_dram_parameter("xhb", [128, GP], BF16, isOutput=False)
    attd = nc.declare_dram_parameter("attb", [48, GP], BF16, isOutput=False)
    smtd = nc.declare_dram_parameter("smt", [128, NSTAT * 128], BF16,
                                     isOutput=False)
    outd = nc.declare_dram_parameter("out", [2, 60, GP], BF16, isOutput=True)

    with TileContext(nc) as tc:
        with (
            tc.tile_pool(name="const", bufs=1) as cpool,
            tc.tile_pool(name="cat", bufs=2) as catp,
            tc.tile_pool(name="hsb", bufs=3) as hsp,
            tc.tile_pool(name="chain", bufs=2) as chp,
            tc.tile_pool(name="msg", bufs=2) as msgp,
            tc.tile_pool(name="gate", bufs=2) as gatep,
            tc.tile_pool(name="outp", bufs=2) as outp,
        ):
            smt = cpool.tile([128, NSTAT * 128], BF16, name="smt")
            nc.sync.dma_start(out=smt[:, :], in_=smtd[:, :])
            attt = cpool.tile([48, GP], BF16, name="attt")
            nc.sync.dma_start(out=attt[:, :], in_=attd[:, :])

            def stat(i, K=128):
                return smt[0:K, i * 128:i * 128 + 128]

            def mm(ps, sidx, rhs_ap, start, stop, K=128):
                nc.tensor.matmul(ps, stat(sidx, K), rhs_ap,
                                 start=start, stop=stop)

            # ---- chunk-0 loads, split across queues for a fast start ----
            def load_chunk(j, engs):
                t = {}
                names = [f"c{i}" for i in range(6)] + ["d0", "d1"]
                srcs = [catc[i] for i in range(6)] + [catd[0], catd[1]]
                for n, (name, src) in enumerate(zip(names, srcs)):
                    tl = catp.tile([128, CW], BF16, tag=name,
                                   name=f"{name}_{j}")
                    engs[n % len(engs)].dma_start(
                        out=tl[:, :], in_=src[:, j * CW:(j + 1) * CW])
                    t[name] = tl
                return t

            tiles = {0: load_chunk(0, [nc.scalar, nc.gpsimd])}
            # xh planar (both halves), whole image, on ACT queue
            xht = cpool.tile([128, GP], BF16, name="xht")
            nc.scalar.dma_start(out=xht[:, :], in_=xhbd[:, :])

            # ---- attention broadcasts for all chunks (PSUM freed after) ----
            buts = [cpool.tile([128, CW], BF16, name=f"buts{j}")
                    for j in range(NCHUNK)]
            blts = [cpool.tile([128, CW], BF16, name=f"blts{j}")
                    for j in range(NCHUNK)]
            with tc.tile_pool(name="attpp", bufs=2, space="PSUM") as attpp:
                for j in range(NCHUNK):
                    for sidx, dst, nm in ((S_BU, buts[j], "pul"),
                                          (S_BL, blts[j], "pa")):
                        ps = attpp.tile([128, CW], F32, tag="aps",
                                        name=f"{nm}{j}")
                        for s in range(0, CW, 512):
                            mm(ps[0:128, s:s + 512], sidx,
                               attt[0:48, j * CW + s:j * CW + s + 512],
                               True, True, K=48)
                        if nm == "pul":
                            nc.scalar.activation(dst[:, :], ps[:, :], AF.Copy)
                        else:
                            nc.vector.tensor_copy(dst[:, :], ps[:, :])

            tiles[1] = load_chunk(1, [nc.sync])

            # ---- main loop ----
            # PSUM: hpp 2x[128,1024] (H psums + GRU gate psums) = 4 banks,
            # zpp 2x[128,1024] (conv2/message psums) = 4 banks.
            with (
                tc.tile_pool(name="hpp", bufs=2, space="PSUM") as hpp,
                tc.tile_pool(name="zpp", bufs=2, space="PSUM") as zpp,
            ):
                for j in range(NCHUNK):
                    if j + 2 < NCHUNK:
                        tiles[j + 2] = load_chunk(j + 2, [nc.sync])
                    t = tiles.pop(j)
                    xsl = xht[:, j * CW:(j + 1) * CW]

                    # premultiplies on Pool (in-place on the cat tiles)
                    for i in range(6):
                        src = t[f"c{i}"]
                        attv = (buts[j][64:124, :] if i < 4
                                else blts[j][64:124, :])
                        nc.gpsimd.tensor_tensor(src[64:124, :], src[64:124, :],
                                                attv, AL.mult)
                    nc.gpsimd.tensor_tensor(t["d0"][0:60, :], t["d0"][0:60, :],
                                            buts[j][0:60, :], AL.mult)
                    nc.gpsimd.tensor_tensor(t["d1"][0:60, :], t["d1"][0:60, :],
                                            blts[j][0:60, :], AL.mult)

                    # conv blocks + message chain (chain on DVE, psum-legal)
                    schain = None
                    relu_k = 0
                    for zc, pair in enumerate(PAIRS):
                        zt = zpp.tile([128, CW], F32, tag="z",
                                      name=f"z{j}_{zc}")
                        for e, (key, w1, w2) in enumerate(pair):
                            hps = hpp.tile([128, CW], F32, tag="h",
                                           name=f"h{j}{key}")
                            for s in range(0, CW, 512):
                                mm(hps[0:128, s:s + 512], w1,
                                   t[key][0:128, s:s + 512], True, True)
                            hsb = hsp.tile([128, CW], BF16, tag="hs",
                                           name=f"hs{j}{key}")
                            if relu_k % 4 == 3:
                                nc.vector.tensor_scalar_max(
                                    hsb[:, :], hps[:, :], 0.0)
                            else:
                                nc.scalar.activation(hsb[:, :], hps[:, :],
                                                     AF.Relu)
                            relu_k += 1
                            for s in range(0, CW, 512):
                                mm(zt[0:128, s:s + 512], w2,
                                   hsb[0:128, s:s + 512],
                                   e == 0, e == len(pair) - 1)
                        if zc == 0:
                            so = chp.tile([128, CW], F32, tag="s",
                                          name=f"s{j}_{zc}")
                            nc.vector.tensor_scalar_max(
                                so[:, :], zt[:, :], 0.0)
                        elif zc < len(PAIRS) - 1:
                            so = chp.tile([128, CW], F32, tag="s",
                                          name=f"s{j}_{zc}")
                            nc.vector.scalar_tensor_tensor(
                                so[:, :], zt[:, :], 0.0, schain[:, :],
                                AL.max, AL.add)
                        else:
                            msgt = msgp.tile([128, CW], BF16, tag="msg",
                                             name=f"msg{j}")
                            so = msgt
                            nc.vector.scalar_tensor_tensor(
                                msgt[:, :], zt[:, :], 0.0, schain[:, :],
                                AL.max, AL.add)
                        schain = so

                    # GRU gates (gate psums share the H psum pool)
                    rt = gatep.tile([128, CW], BF16, tag="rt", name=f"rt{j}")
                    ut = gatep.tile([128, CW], BF16, tag="ut", name=f"ut{j}")
                    ct = gatep.tile([128, CW], BF16, tag="ct", name=f"ct{j}")
                    for sm, sh, dst, fn in ((S_GRM, S_GRH, rt, AF.Sigmoid),
                                            (S_GUM, S_GUH, ut, AF.Sigmoid)):
                        pg = hpp.tile([128, CW], F32, tag="h",
                                      name=f"g{j}{sm}")
                        for s in range(0, CW, 512):
                            mm(pg[0:128, s:s + 512], sm,
                               msgt[0:128, s:s + 512], True, False)
                            mm(pg[0:128, s:s + 512], sh,
                               xht[0:128, j * CW + s:j * CW + s + 512],
                               False, True)
                        nc.scalar.activation(dst[:, :], pg[:, :], fn)
                    rht = gatep.tile([128, CW], BF16, tag="rh", name=f"rh{j}")
                    nc.gpsimd.tensor_tensor(rht[:, :], rt[:, :], xsl, AL.mult)
                    pg = hpp.tile([128, CW], F32, tag="h", name=f"gc{j}")
                    for s in range(0, CW, 512):
                        mm(pg[0:128, s:s + 512], S_GCM,
                           msgt[0:128, s:s + 512], True, False)
                        mm(pg[0:128, s:s + 512], S_GCRH,
                           rht[0:128, s:s + 512], False, True)
                    nc.scalar.activation(ct[:, :], pg[:, :], AF.Tanh)

                    # combine: out = h + u*(c - h)
                    dt = gatep.tile([128, CW], BF16, tag="dt", name=f"dt{j}")
                    nc.gpsimd.tensor_tensor(dt[:, :], ct[:, :], xsl,
                                            AL.subtract)
                    et = gatep.tile([128, CW], BF16, tag="et", name=f"et{j}")
                    nc.gpsimd.tensor_tensor(et[:, :], ut[:, :], dt[:, :],
                                            AL.mult)
                    ot = outp.tile([128, CW], BF16, tag="ot", name=f"ot{j}")
                    nc.vector.tensor_tensor(ot[:, :], xsl, et[:, :], AL.add)

                    nc.sync.dma_start(out=outd[0, :, j * CW:(j + 1) * CW],
                                      in_=ot[0:60, :])
                    nc.sync.dma_start(out=outd[1, :, j * CW:(j + 1) * CW],
                                      in_=ot[64:124, :])

    nc.compile()
    return nc


def _fold(W, p):
    g, b, m, v = p[0], p[1], p[2], p[3]
    s = g / np.sqrt(v + EPS)
    return (s[:, None] * W).astype(np.float32), (b - m * s).astype(np.float32)


def _build_stats(dW1, dbn1, dW2, dbn2, uW1, ubn1, uW2, ubn2,
                 lW1, lbn1, lW2, lbn2, guWg, gubg, guWc, gubc,
                 glWg, glbg, glWc, glbc):
    dW1f, bd1 = _fold(dW1, dbn1)
    dW2f, bd2 = _fold(dW2, dbn2)
    uW1f, bu1 = _fold(uW1, ubn1)
    uW2f, bu2 = _fold(uW2, ubn2)
    lW1f, bl1 = _fold(lW1, lbn1)
    lW2f, bl2 = _fold(lW2, lbn2)

    S = np.zeros((NSTAT, 128, 128), np.float32)
    for g in range(G):
        r = 10 * g
        S[S_BU, g, r:r + 10] = 1.0          # h_att1 -> rows 0:60
        for k in (1, 2, 3, 4):              # sum p_att1..4 -> rows 64:124
            S[S_BU, 12 + 6 * (k - 1) + g, 64 + r:64 + r + 10] = 1.0
        S[S_BL, 6 + g, r:r + 10] = 1.0      # h_att2 -> rows 0:60
        for k in (5, 6):                    # sum p_att5..6 -> rows 64:124
            S[S_BL, 12 + 6 * (k - 1) + g, 64 + r:64 + r + 10] = 1.0

    def conv1(idx, Wf, bias):
        # cat rows [0:60]=first input (in-ch 0..9), [64:124]=second (10..19)
        for g in range(G):
            r = 10 * g
            S[idx, r:r + 10, r:r + 10] = Wf[0:10, 0:10].T
            S[idx, r:r + 10, 64 + r:64 + r + 10] = Wf[10:20, 0:10].T
            S[idx, 64 + r:64 + r + 10, r:r + 10] = Wf[0:10, 10:20].T
            S[idx, 64 + r:64 + r + 10, 64 + r:64 + r + 10] = Wf[10:20, 10:20].T
            S[idx, 60, r:r + 10] = bias[0:10]
            S[idx, 60, 64 + r:64 + r + 10] = bias[10:20]
        S[idx, 60, 60] = 1.0    # H ones-row for conv2 bias injection

    conv1(S_CD, dW1f, bd1)
    conv1(S_CU, uW1f, bu1)
    conv1(S_CL, lW1f, bl1)

    def conv2(idx, Wf, bias, off, ones):
        for g in range(G):
            r = 10 * g
            S[idx, r:r + 10, off + r:off + r + 10] = Wf[:, 0:10].T
            S[idx, 64 + r:64 + r + 10, off + r:off + r + 10] = Wf[:, 10:20].T
            S[idx, 60, off + r:off + r + 10] = bias
        if ones:
            S[idx, 60, 60] = 1.0    # msg ones-row for GRU bias injection

    conv2(S_ZDA, dW2f, bd2, 0, True)
    conv2(S_ZDB, dW2f, bd2, 64, False)
    conv2(S_ZUA, uW2f, bu2, 0, False)
    conv2(S_ZUB, uW2f, bu2, 64, False)
    conv2(S_ZLB, lW2f, bl2, 64, False)

    def gru(idx, Wu, Wl, rows, incol, bu_, bl_):
        for g in range(G):
            r = 10 * g
            S[idx, r:r + 10, r:r + 10] = Wu[rows, incol:incol + 10].T
            S[idx, 64 + r:64 + r + 10, 64 + r:64 + r + 10] = \
                Wl[rows, incol:incol + 10].T
            if bu_ is not None:
                S[idx, 60, r:r + 10] = bu_
                S[idx, 60, 64 + r:64 + r + 10] = bl_

    gru(S_GRM, guWg, glWg, slice(0, 10), 0, gubg[0:10], glbg[0:10])
    gru(S_GRH, guWg, glWg, slice(0, 10), 10, None, None)
    gru(S_GUM, guWg, glWg, slice(10, 20), 0, gubg[10:20], glbg[10:20])
    gru(S_GUH, guWg, glWg, slice(10, 20), 10, None, None)
    gru(S_GCM, guWc, glWc, slice(0, 10), 0, gubc, glbc)
    gru(S_GCRH, guWc, glWc, slice(0, 10), 10, None, None)
    return S


BF_NP = mybir.dt.np(mybir.dt.bfloat16)


def _planar(a):
    # [HD, H, W] -> [60, GP]: row 10*g + c
    a = np.asarray(a, np.float32).reshape(HD, G, GP)
    return np.moveaxis(a, 1, 0).reshape(60, GP)


def _unplanar(a):
    # [2, 60, GP] -> [2, HD, 192, 192]
    a = a.reshape(2, G, HD, GP)
    return np.moveaxis(a, 1, 2).reshape(2, HD, 192, 192)


def make_in_maps(xf, xh, xp, h_att, p_att, S):
    smt = np.ascontiguousarray(
        S.transpose(1, 0, 2).reshape(128, NSTAT * 128)).astype(BF_NP)
    in_maps = []
    for b in range(B):
        xfp = _planar(xf[b])
        xhu = _planar(xh[0, b])
        xhl = _planar(xh[1, b])
        catc = np.zeros((6, 128, GP), np.float32)
        for i in range(6):
            catc[i, 0:60] = xhu if i < 4 else xhl
            catc[i, 60] = 1.0
            catc[i, 64:124] = _planar(xp[i, b])
        catd = np.zeros((2, 128, GP), np.float32)
        for i, xh_half in enumerate((xhu, xhl)):
            catd[i, 0:60] = xfp
            catd[i, 60] = 1.0
            catd[i, 64:124] = xh_half
        xhb = np.zeros((128, GP), np.float32)
        xhb[0:60] = xhu
        xhb[64:124] = xhl
        attb = np.zeros((48, GP), np.float32)
        attb[0:6] = h_att[1, b, 0].reshape(G, GP)
        attb[6:12] = h_att[2, b, 0].reshape(G, GP)
        for k in range(1, 7):
            attb[12 + 6 * (k - 1):12 + 6 * k] = p_att[k, b, 0].reshape(G, GP)
        in_maps.append(dict(
            catc=np.ascontiguousarray(catc).astype(BF_NP),
            catd=np.ascontiguousarray(catd).astype(BF_NP),
            xhb=np.ascontiguousarray(xhb).astype(BF_NP),
            attb=np.ascontiguousarray(attb).astype(BF_NP),
            smt=smt,
        ))
    return in_maps


_NC_CACHE = None


def _get_nc():
    global _NC_CACHE
    if _NC_CACHE is None:
        _NC_CACHE = _build_nc()
    return _NC_CACHE


def _prep(xf, xh, xp, h_att, p_att,
          dW1, dbn1, dW2, dbn2, uW1, ubn1, uW2, ubn2,
          lW1, lbn1, lW2, lbn2, guWg, gubg, guWc, gubc,
          glWg, glbg, glWc, glbc):
    args = [np.asarray(a, dtype=np.float32) for a in
            (dW1, dbn1, dW2, dbn2, uW1, ubn1, uW2, ubn2,
             lW1, lbn1, lW2, lbn2, guWg, gubg, guWc, gubc,
             glWg, glbg, glWc, glbc)]
    S = _build_stats(*args)
    return make_in_maps(np.asarray(xf, np.float32), np.asarray(xh, np.float32),
                        np.asarray(xp, np.float32),
                        np.asarray(h_att, np.float32),
                        np.asarray(p_att, np.float32), S)


def kernel(xf, xh, xp, h_att, p_att,
           dW1, dbn1, dW2, dbn2,
           uW1, ubn1, uW2, ubn2,
           lW1, lbn1, lW2, lbn2,
           guWg, gubg, guWc, gubc,
           glWg, glbg, glWc, glbc,
           _trace=False):
    from concourse.bass_utils import run_bass_kernel_spmd

    in_maps = _prep(xf, xh, xp, h_att, p_att,
                    dW1, dbn1, dW2, dbn2, uW1, ubn1, uW2, ubn2,
                    lW1, lbn1, lW2, lbn2, guWg, gubg, guWc, gubc,
                    glWg, glbg, glWc, glbc)
    nc = _get_nc()
    res = run_bass_kernel_spmd(nc, in_maps, core_ids=list(range(B)),
                               trace=_trace)
    out = np.empty((2, B, HD, 192, 192), np.float32)
    for b in range(B):
        out[:, b] = _unplanar(np.asarray(res.results[b]["out"], np.float32))
    if _trace:
        return out, res
    return out


# revision 12
# speedup vs baseline: 1.8207x; 1.2921x over previous
"""Trainium2 Bass kernel for nn_Half_Graph (GNN message passing block).

Data-parallel over batch: core b processes image b (B=8 across 8 cores).

Planar layout: SBUF partition 10*g + c <-> (channel c, pixel group g) with
G=6 groups of 6144 pixels; a 10-channel tensor occupies 60 partitions.
128-partition tiles hold two such 60-row halves at [0:60] and [64:124].

All convs are 1x1 -> matmuls with block-diagonal stationaries. Host builds
bf16 concat tensors [first; second] per conv block so each conv1 is a
SINGLE matmul pass (K=128). Partition row 60 of every concat tensor is a
constant 1.0, and stationary row 60 carries the folded BN / GRU biases, so
biases ride along in the matmul for free. The attention premultiply reads
a PE-broadcast copy of the attention maps (computed once for all chunks up
front). The inter-block message sum is a relu+add chain on Pool/DVE
reading conv2 PSUM tiles directly. GRU output uses out = h + u*(c - h).

Host side pre-transposes/concats/casts (cheap, not part of measured
device time) so every DMA is a plain 2D bf16 slice.
"""

import sys

for _p in ("/opt/trn_rl_repo", "/root/.axon_site/_ro/trn_rl_repo"):
    if _p not in sys.path:
        sys.path.insert(0, _p)

import numpy as np

import concourse.bass as bass
import concourse.bacc as bacc
import concourse.mybir as mybir
from concourse.tile import TileContext

F32 = mybir.dt.float32
BF16 = mybir.dt.bfloat16
AL = mybir.AluOpType
AF = mybir.ActivationFunctionType

B = 8
HD = 10
G = 6
HW = 192 * 192          # 36864 pixels
GP = HW // G            # 6144 pixels per group
CW = 1024               # chunk width (columns per group per chunk)
NCHUNK = GP // CW       # 6 chunks
EPS = 1e-5

NSTAT = 16
# S_BU broadcast: [h_att1 (rows 0:60); sum p_att1..4 (rows 64:124)]
# S_BL broadcast: [h_att2 (rows 0:60); sum p_att5..6 (rows 64:124)]
(S_BU, S_BL, S_CD, S_CU, S_CL,
 S_ZDA, S_ZDB, S_ZUA, S_ZUB, S_ZLB,
 S_GRM, S_GRH, S_GUM, S_GUH, S_GCM, S_GCRH) = range(NSTAT)

# conv block schedule: pairs of (cat-tile key, conv1 stat, conv2 stat).
# Each pair accumulates into one Z psum: first element -> z cols [0:60],
# second -> z cols [64:124]. Pairs 3/4 are single (odd number of upper z's).
PAIRS = [
    [("d0", S_CD, S_ZDA), ("d1", S_CD, S_ZDB)],
    [("c0", S_CU, S_ZUA), ("c4", S_CL, S_ZLB)],
    [("c1", S_CU, S_ZUA), ("c5", S_CL, S_ZLB)],
    [("c2", S_CU, S_ZUA)],
    [("c3", S_CU, S_ZUA)],
]


def _build_nc():
    nc = bacc.Bacc(trn_type="TRN2")

    catc = nc.declare_dram_parameter("catc", [6, 128, GP], BF16, isOutput=False)
    catd = nc.declare_dram_parameter("catd", [2, 128, GP], BF16, isOutput=False)
    xhbd = nc.declare_dram_parameter("xhb", [128, GP], BF16, isOutput=False)
    attd = nc.declare_dram_parameter("attb", [48, GP], BF16, isOutput=False)
    smtd = nc.declare_dram_parameter("smt", [128, NSTAT * 128], BF16,
                                     isOutput=False)
    outd = nc.declare_dram_parameter("out", [2, 60, GP], BF16, isOutput=True)

    with TileContext(nc) as tc:
        with (
            tc.tile_pool(name="const", bufs=1) as cpool,
            tc.tile_pool(name="cat", bufs=2) as catp,
            tc.tile_pool(name="hsb", bufs=3) as hsp,
            tc.tile_pool(name="chain", bufs=2) as chp,
            tc.tile_pool(name="msg", bufs=2) as msgp,
            tc.tile_pool(name="gate", bufs=2) as gatep,
            tc.tile_pool(name="outp", bufs=2) as outp,
        ):
            smt = cpool.tile([128, NSTAT * 128], BF16, name="smt")
            nc.sync.dma_start(out=smt[:, :], in_=smtd[:, :])
            atts = []
            for j in range(NCHUNK):
                a = cpool.tile([48, CW], BF16, name=f"att{j}")
                nc.sync.dma_start(out=a[:, :],
                                  in_=attd[:, j * CW:(j + 1) * CW])
                atts.append(a)

            def stat(i, K=128):
                return smt[0:K, i * 128:i * 128 + 128]

            def mm(ps, sidx, rhs_ap, start, stop, K=128):
                nc.tensor.matmul(ps, stat(sidx, K), rhs_ap,
                                 start=start, stop=stop)

            def load_chunk(j, engs):
                t = {}
                names = [f"c{i}" for i in range(6)] + ["d0", "d1"]
                srcs = [catc[i] for i in range(6)] + [catd[0], catd[1]]
                for n, (name, src) in enumerate(zip(names, srcs)):
                    tl = catp.tile([128, CW], BF16, tag=name,
                                   name=f"{name}_{j}")
                    engs[n % len(engs)].dma_start(
                        out=tl[:, :], in_=src[:, j * CW:(j + 1) * CW])
                    t[name] = tl
                return t

            # chunk-0 loads split across the two idle-at-start queues;
            # xh planar (needed first by tail(0)) behind Pool's share
            tiles = {0: load_chunk(0, [nc.scalar, nc.gpsimd])}
            xht = cpool.tile([128, GP], BF16, name="xht")
            nc.gpsimd.dma_start(out=xht[:, :], in_=xhbd[:, :])

            # ---- attention broadcasts for all chunks (PSUM freed after) ----
            buts = [cpool.tile([128, CW], BF16, name=f"buts{j}")
                    for j in range(NCHUNK)]
            blts = [cpool.tile([128, CW], BF16, name=f"blts{j}")
                    for j in range(NCHUNK)]
            with tc.tile_pool(name="attpp", bufs=2, space="PSUM") as attpp:
                for j in range(NCHUNK):
                    for sidx, dst, nm in ((S_BU, buts[j], "pul"),
                                          (S_BL, blts[j], "pa")):
                        ps = attpp.tile([128, CW], F32, tag="aps",
                                        name=f"{nm}{j}")
                        for s in range(0, CW, 512):
                            mm(ps[0:128, s:s + 512], sidx,
                               atts[j][0:48, s:s + 512], True, True, K=48)
                        if nm == "pul":
                            nc.scalar.activation(dst[:, :], ps[:, :], AF.Copy)
                        else:
                            nc.vector.tensor_copy(dst[:, :], ps[:, :])

            # ---- software-pipelined main loop ----
            # PSUM: hpp 2x[128,1024]=4 banks (H), zpp 1x[128,1024]=2 (Z),
            # gpp 1x[128,1024]=2 (GRU gates). front(j+1) is issued BEFORE
            # tail(j) on every queue so the serial GRU tail of chunk j
            # overlaps the conv front of chunk j+1.
            with (
                tc.tile_pool(name="hpp", bufs=2, space="PSUM") as hpp,
                tc.tile_pool(name="zpp", bufs=1, space="PSUM") as zpp,
                tc.tile_pool(name="gpp", bufs=1, space="PSUM") as gpp,
            ):
                msgts = {}

                def front(j):
                    if j + 1 < NCHUNK:
                        tiles[j + 1] = load_chunk(j + 1, [nc.sync])
                    t = tiles.pop(j)

                    # premultiplies on Pool (in-place on the cat tiles)
                    for i in range(6):
                        src = t[f"c{i}"]
                        attv = (buts[j][64:124, :] if i < 4
                                else blts[j][64:124, :])
                        nc.gpsimd.tensor_tensor(src[64:124, :], src[64:124, :],
                                                attv, AL.mult)
                    nc.gpsimd.tensor_tensor(t["d0"][0:60, :], t["d0"][0:60, :],
                                            buts[j][0:60, :], AL.mult)
                    nc.gpsimd.tensor_tensor(t["d1"][0:60, :], t["d1"][0:60, :],
                                            blts[j][0:60, :], AL.mult)

                    # conv blocks + message chain (chain on DVE, psum-legal)
                    schain = None
                    relu_k = 0
                    for zc, pair in enumerate(PAIRS):
                        zt = zpp.tile([128, CW], F32, tag="z",
                                      name=f"z{j}_{zc}")
                        for e, (key, w1, w2) in enumerate(pair):
                            hps = hpp.tile([128, CW], F32, tag="h",
                                           name=f"h{j}{key}")
                            for s in range(0, CW, 512):
                                mm(hps[0:128, s:s + 512], w1,
                                   t[key][0:128, s:s + 512], True, True)
                            hsb = hsp.tile([128, CW], BF16, tag="hs",
                                           name=f"hs{j}{key}")
                            if relu_k % 4 == 3:
                                nc.vector.tensor_scalar_max(
                                    hsb[:, :], hps[:, :], 0.0)
                            else:
                                nc.scalar.activation(hsb[:, :], hps[:, :],
                                                     AF.Relu)
                            relu_k += 1
                            for s in range(0, CW, 512):
                                mm(zt[0:128, s:s + 512], w2,
                                   hsb[0:128, s:s + 512],
                                   e == 0, e == len(pair) - 1)
                        if zc == 0:
                            so = chp.tile([128, CW], F32, tag="s",
                                          name=f"s{j}_{zc}")
                            nc.vector.tensor_scalar_max(
                                so[:, :], zt[:, :], 0.0)
                        elif zc < len(PAIRS) - 1:
                            so = chp.tile([128, CW], F32, tag="s",
                                          name=f"s{j}_{zc}")
                            nc.vector.scalar_tensor_tensor(
                                so[:, :], zt[:, :], 0.0, schain[:, :],
                                AL.max, AL.add)
                        else:
                            so = msgp.tile([128, CW], BF16, tag="msg",
                                           name=f"msg{j}")
                            nc.vector.scalar_tensor_tensor(
                                so[:, :], zt[:, :], 0.0, schain[:, :],
                                AL.max, AL.add)
                            msgts[j] = so
                        schain = so

                def tail(j):
                    msgt = msgts.pop(j)
                    xsl = xht[:, j * CW:(j + 1) * CW]
                    rt = gatep.tile([128, CW], BF16, tag="rt", name=f"rt{j}")
                    ut = gatep.tile([128, CW], BF16, tag="ut", name=f"ut{j}")
                    ct = gatep.tile([128, CW], BF16, tag="ct", name=f"ct{j}")
                    for sm, sh, dst, fn in ((S_GRM, S_GRH, rt, AF.Sigmoid),
                                            (S_GUM, S_GUH, ut, AF.Sigmoid)):
                        pg = gpp.tile([128, CW], F32, tag="g",
                                      name=f"g{j}{sm}")
                        for s in range(0, CW, 512):
                            mm(pg[0:128, s:s + 512], sm,
                               msgt[0:128, s:s + 512], True, False)
                            mm(pg[0:128, s:s + 512], sh,
                               xht[0:128, j * CW + s:j * CW + s + 512],
                               False, True)
                        nc.scalar.activation(dst[:, :], pg[:, :], fn)
                    rht = gatep.tile([128, CW], BF16, tag="rh", name=f"rh{j}")
                    nc.gpsimd.tensor_tensor(rht[:, :], rt[:, :], xsl, AL.mult)
                    pg = gpp.tile([128, CW], F32, tag="g", name=f"gc{j}")
                    for s in range(0, CW, 512):
                        mm(pg[0:128, s:s + 512], S_GCM,
                           msgt[0:128, s:s + 512], True, False)
                        mm(pg[0:128, s:s + 512], S_GCRH,
                           rht[0:128, s:s + 512], False, True)
                    nc.scalar.activation(ct[:, :], pg[:, :], AF.Tanh)

                    # combine: out = h + u*(c - h)
                    dt = gatep.tile([128, CW], BF16, tag="dt", name=f"dt{j}")
                    nc.gpsimd.tensor_tensor(dt[:, :], ct[:, :], xsl,
                                            AL.subtract)
                    et = gatep.tile([128, CW], BF16, tag="et", name=f"et{j}")
                    nc.gpsimd.tensor_tensor(et[:, :], ut[:, :], dt[:, :],
                                            AL.mult)
                    ot = outp.tile([128, CW], BF16, tag="ot", name=f"ot{j}")
                    nc.vector.tensor_tensor(ot[:, :], xsl, et[:, :], AL.add)

                    nc.sync.dma_start(out=outd[0, :, j * CW:(j + 1) * CW],
                                      in_=ot[0:60, :])
                    nc.sync.dma_start(out=outd[1, :, j * CW:(j + 1) * CW],
                                      in_=ot[64:124, :])

                front(0)
                for j in range(NCHUNK):
                    if j + 1 < NCHUNK:
                        front(j + 1)
                    tail(j)

    nc.compile()
    return nc


def _fold(W, p):
    g, b, m, v = p[0], p[1], p[2], p[3]
    s = g / np.sqrt(v + EPS)
    return (s[:, None] * W).astype(np.float32), (b - m * s).astype(np.float32)


def _build_stats(dW1, dbn1, dW2, dbn2, uW1, ubn1, uW2, ubn2,
                 lW1, lbn1, lW2, lbn2, guWg, gubg, guWc, gubc,
                 glWg, glbg, glWc, glbc):
    dW1f, bd1 = _fold(dW1, dbn1)
    dW2f, bd2 = _fold(dW2, dbn2)
    uW1f, bu1 = _fold(uW1, ubn1)
    uW2f, bu2 = _fold(uW2, ubn2)
    lW1f, bl1 = _fold(lW1, lbn1)
    lW2f, bl2 = _fold(lW2, lbn2)

    S = np.zeros((NSTAT, 128, 128), np.float32)
    for g in range(G):
        r = 10 * g
        S[S_BU, g, r:r + 10] = 1.0          # h_att1 -> rows 0:60
        for k in (1, 2, 3, 4):              # sum p_att1..4 -> rows 64:124
            S[S_BU, 12 + 6 * (k - 1) + g, 64 + r:64 + r + 10] = 1.0
        S[S_BL, 6 + g, r:r + 10] = 1.0      # h_att2 -> rows 0:60
        for k in (5, 6):                    # sum p_att5..6 -> rows 64:124
            S[S_BL, 12 + 6 * (k - 1) + g, 64 + r:64 + r + 10] = 1.0

    def conv1(idx, Wf, bias):
        # cat rows [0:60]=first input (in-ch 0..9), [64:124]=second (10..19)
        for g in range(G):
            r = 10 * g
            S[idx, r:r + 10, r:r + 10] = Wf[0:10, 0:10].T
            S[idx, r:r + 10, 64 + r:64 + r + 10] = Wf[10:20, 0:10].T
            S[idx, 64 + r:64 + r + 10, r:r + 10] = Wf[0:10, 10:20].T
            S[idx, 64 + r:64 + r + 10, 64 + r:64 + r + 10] = Wf[10:20, 10:20].T
            S[idx, 60, r:r + 10] = bias[0:10]
            S[idx, 60, 64 + r:64 + r + 10] = bias[10:20]
        S[idx, 60, 60] = 1.0    # H ones-row for conv2 bias injection

    conv1(S_CD, dW1f, bd1)
    conv1(S_CU, uW1f, bu1)
    conv1(S_CL, lW1f, bl1)

    def conv2(idx, Wf, bias, off, ones):
        for g in range(G):
            r = 10 * g
            S[idx, r:r + 10, off + r:off + r + 10] = Wf[:, 0:10].T
            S[idx, 64 + r:64 + r + 10, off + r:off + r + 10] = Wf[:, 10:20].T
            S[idx, 60, off + r:off + r + 10] = bias
        if ones:
            S[idx, 60, 60] = 1.0    # msg ones-row for GRU bias injection

    conv2(S_ZDA, dW2f, bd2, 0, True)
    conv2(S_ZDB, dW2f, bd2, 64, False)
    conv2(S_ZUA, uW2f, bu2, 0, False)
    conv2(S_ZUB, uW2f, bu2, 64, False)
    conv2(S_ZLB, lW2f, bl2, 64, False)

    def gru(idx, Wu, Wl, rows, incol, bu_, bl_):
        for g in range(G):
            r = 10 * g
            S[idx, r:r + 10, r:r + 10] = Wu[rows, incol:incol + 10].T
            S[idx, 64 + r:64 + r + 10, 64 + r:64 + r + 10] = \
                Wl[rows, incol:incol + 10].T
            if bu_ is not None:
                S[idx, 60, r:r + 10] = bu_
                S[idx, 60, 64 + r:64 + r + 10] = bl_

    gru(S_GRM, guWg, glWg, slice(0, 10), 0, gubg[0:10], glbg[0:10])
    gru(S_GRH, guWg, glWg, slice(0, 10), 10, None, None)
    gru(S_GUM, guWg, glWg, slice(10, 20), 0, gubg[10:20], glbg[10:20])
    gru(S_GUH, guWg, glWg, slice(10, 20), 10, None, None)
    gru(S_GCM, guWc, glWc, slice(0, 10), 0, gubc, glbc)
    gru(S_GCRH, guWc, glWc, slice(0, 10), 10, None, None)
    return S


BF_NP = mybir.dt.np(mybir.dt.bfloat16)


def _planar(a):
    # [HD, H, W] -> [60, GP]: row 10*g + c
    a = np.asarray(a, np.float32).reshape(HD, G, GP)
    return np.moveaxis(a, 1, 0).reshape(60, GP)


def _unplanar(a):
    # [2, 60, GP] -> [2, HD, 192, 192]
    a = a.reshape(2, G, HD, GP)
    return np.moveaxis(a, 1, 2).reshape(2, HD, 192, 192)


def make_in_maps(xf, xh, xp, h_att, p_att, S):
    smt = np.ascontiguousarray(
        S.transpose(1, 0, 2).reshape(128, NSTAT * 128)).astype(BF_NP)
    in_maps = []
    for b in range(B):
        xfp = _planar(xf[b])
        xhu = _planar(xh[0, b])
        xhl = _planar(xh[1, b])
        catc = np.zeros((6, 128, GP), np.float32)
        for i in range(6):
            catc[i, 0:60] = xhu if i < 4 else xhl
            catc[i, 60] = 1.0
            catc[i, 64:124] = _planar(xp[i, b])
        catd = np.zeros((2, 128, GP), np.float32)
        for i, xh_half in enumerate((xhu, xhl)):
            catd[i, 0:60] = xfp
            catd[i, 60] = 1.0
            catd[i, 64:124] = xh_half
        xhb = np.zeros((128, GP), np.float32)
        xhb[0:60] = xhu
        xhb[64:124] = xhl
        attb = np.zeros((48, GP), np.float32)
        attb[0:6] = h_att[1, b, 0].reshape(G, GP)
        attb[6:12] = h_att[2, b, 0].reshape(G, GP)
        for k in range(1, 7):
            attb[12 + 6 * (k - 1):12 + 6 * k] = p_att[k, b, 0].reshape(G, GP)
        in_maps.append(dict(
            catc=np.ascontiguousarray(catc).astype(BF_NP),
            catd=np.ascontiguousarray(catd).astype(BF_NP),
            xhb=np.ascontiguousarray(xhb).astype(BF_NP),
            attb=np.ascontiguousarray(attb).astype(BF_NP),
            smt=smt,
        ))
    return in_maps


_NC_CACHE = None


def _get_nc():
    global _NC_CACHE
    if _NC_CACHE is None:
        _NC_CACHE = _build_nc()
    return _NC_CACHE


def _prep(xf, xh, xp, h_att, p_att,
          dW1, dbn1, dW2, dbn2, uW1, ubn1, uW2, ubn2,
          lW1, lbn1, lW2, lbn2, guWg, gubg, guWc, gubc,
          glWg, glbg, glWc, glbc):
    args = [np.asarray(a, dtype=np.float32) for a in
            (dW1, dbn1, dW2, dbn2, uW1, ubn1, uW2, ubn2,
             lW1, lbn1, lW2, lbn2, guWg, gubg, guWc, gubc,
             glWg, glbg, glWc, glbc)]
    S = _build_stats(*args)
    return make_in_maps(np.asarray(xf, np.float32), np.asarray(xh, np.float32),
                        np.asarray(xp, np.float32),
                        np.asarray(h_att, np.float32),
                        np.asarray(p_att, np.float32), S)


def kernel(xf, xh, xp, h_att, p_att,
           dW1, dbn1, dW2, dbn2,
           uW1, ubn1, uW2, ubn2,
           lW1, lbn1, lW2, lbn2,
           guWg, gubg, guWc, gubc,
           glWg, glbg, glWc, glbc,
           _trace=False):
    from concourse.bass_utils import run_bass_kernel_spmd

    in_maps = _prep(xf, xh, xp, h_att, p_att,
                    dW1, dbn1, dW2, dbn2, uW1, ubn1, uW2, ubn2,
                    lW1, lbn1, lW2, lbn2, guWg, gubg, guWc, gubc,
                    glWg, glbg, glWc, glbc)
    nc = _get_nc()
    res = run_bass_kernel_spmd(nc, in_maps, core_ids=list(range(B)),
                               trace=_trace)
    out = np.empty((2, B, HD, 192, 192), np.float32)
    for b in range(B):
        out[:, b] = _unplanar(np.asarray(res.results[b]["out"], np.float32))
    if _trace:
        return out, res
    return out
